# revision 1
# baseline (speedup 1.0000x reference)
"""Distributed causal-attention-with-dropout kernel for 8 TRN2 NeuronCores, v15.

Architecture ("all-local projections", fully static SPMD graph):

- Host pre-formats inputs (layout only, all model FLOPs stay on device):
  each core receives xq = x^T columns of its 4 OWNED q-tiles
  {c, 15-c, 16+c, 31-c} (bf16), the FULL Wq^T / Wk^T / Wv^T (bf16,
  replicated), dropout-mask rows of its owned tiles (bf16), and the causal
  threshold table. There is NO x gather at all.
- A tiny dummy AllGather at t=0 absorbs the ~50us collective-runtime warmup.
- All three projections are LOCAL to the core's own 512 q-rows:
    V = x_own @ Wv^T  (natural [seq, d] layout; stationary = x^T chunks)
    K^T[:, own] = Wk^T.T @ x^T_own   (stationary = Wk^T chunks)
    Q^T[:, own] = Wq^T.T @ x^T_own   (stays in SBUF, no collective)
  V is ki-outer-paced so matmuls start as soon as the first Wv^T strip
  lands. Weights for K/Q load as chunks, pipelined with the dt loop.
- Collectives (serial on the CC core) are only: V AllGather (16MB out) and
  TWO K AllGathers of 8MB each. K^T is written tile-major ([4][2048][128]
  per core), so K-AG chunk 0 = true k-tiles 0..15 and chunk 1 = 16..31:
  attention big-block B needs chunk B//2 only.
- Attention: core c owns q-tiles {c, 15-c, 16+c, 31-c}; k-blocks are 1024
  wide, giving a ZERO-padding static schedule (slot s needs exactly s+1
  blocks; 10 pairs). Causality enforced per-row by (iota(p-j) >= thr) * P
  on the vector engine; softmax without max-subtraction; denominators use
  pre-dropout sums. Pair p's P-transposes and attn@V run after pair p+1's
  score matmuls (software pipeline).
"""

import math
import os
import sys
from contextlib import ExitStack

import numpy as np
import ml_dtypes

for _p in ("/opt/trn_rl_repo", "/root/.axon_site/_ro/trn_rl_repo"):
    if os.path.isdir(_p) and _p not in sys.path:
        sys.path.append(_p)

import concourse.bass as bass
import concourse.tile as tile
from concourse import bacc, mybir
from concourse import bass_utils
from concourse.masks import make_identity

S, D = 4096, 2048
NC = 8
SB = 512          # seq rows per core (4 owned 128-tiles)
BK = 1024         # big k-block width
NBIG = 4
KBMAX = (1, 2, 3, 4)
PBASE = (0, 1, 3, 6)
NPAIR = 10
SCALE = 1.0 / math.sqrt(float(D))
F32 = mybir.dt.float32
BF16 = mybir.dt.bfloat16
RG = [list(range(NC))]
ALU = mybir.AluOpType
AFT = mybir.ActivationFunctionType


def owned_tiles(c):
    return (c, 15 - c, 16 + c, 31 - c)


def tile_owner_slot(t):
    if t <= 7:
        return t, 0
    if t <= 15:
        return 15 - t, 1
    if t <= 23:
        return t - 16, 2
    return 31 - t, 3


# row of tile t inside its V AllGather chunk (chunk = t//16; within a
# chunk, rank blocks of 256 rows hold slots {0,1} or {2,3})
VROW2 = [256 * tile_owner_slot(t)[0] + 128 * (tile_owner_slot(t)[1] % 2)
         for t in range(32)]
# K AllGather block index of tile t: chunk t//16, block 2*c + (s % 2)
KBLK = [2 * tile_owner_slot(t)[0] + (tile_owner_slot(t)[1] % 2)
        for t in range(32)]


def build():
    nc = bacc.Bacc("TRN2", target_bir_lowering=False, debug=False,
                   num_devices=NC)

    xq_in = nc.dram_tensor("xq", [D, SB], BF16, kind="ExternalInput").ap()
    wq_in = nc.dram_tensor("wqT", [D, D], BF16, kind="ExternalInput").ap()
    wv_in = nc.dram_tensor("wvT", [D, D], BF16, kind="ExternalInput").ap()
    wk_in = nc.dram_tensor("wkT", [D, D], BF16, kind="ExternalInput").ap()
    mask_in = nc.dram_tensor("drop_mask", [4 * 128, S], BF16,
                             kind="ExternalInput").ap()
    sched_in = nc.dram_tensor("sched", [128, NPAIR], F32,
                              kind="ExternalInput").ap()
    out_ext = nc.dram_tensor("out", [4 * 128, D], F32,
                             kind="ExternalOutput").ap()

    with tile.TileContext(nc) as tc:
        with ExitStack() as es:
            dram = es.enter_context(tc.tile_pool(name="dram", bufs=1,
                                                 space="DRAM"))
            const = es.enter_context(tc.tile_pool(name="const", bufs=1))
            psum = es.enter_context(tc.tile_pool(name="psum", bufs=1,
                                                 space="PSUM"))

            # ---------------- DRAM scratch ----------------
            dummy_in = dram.tile([1, NPAIR], F32, name="dummy_in")
            dummy_out = dram.tile([NC, NPAIR], F32, addr_space="Shared",
                                  name="dummy_out")
            # V contributions split by slot-pair: chunk 0 = slots {0,1}
            # (true tiles 0..15), chunk 1 = slots {2,3} (tiles 16..31).
            vq_in = [dram.tile([256, D], BF16, name=f"vq_in{h}")
                     for h in range(2)]
            vg = [dram.tile([NC * 256, D], BF16, addr_space="Shared",
                            name=f"vg{h}") for h in range(2)]
            # per-core K^T contribution, 2 dout-halves of [1024, 4x128 q]:
            # chunk h covers dout rows 1024h..1024h+1024 (= score ki half h)
            kq_in = [dram.tile([BK, SB], BF16, name=f"kq_in{h}")
                     for h in range(2)]
            kg = [dram.tile([NC * BK, SB], BF16, addr_space="Shared",
                            name=f"kg{h}") for h in range(2)]

            # ---------------- constants ----------------
            # dummy AllGather first: the CC core takes ~50us to boot after
            # its first trigger, and the first real AG now fires at ~65us —
            # absorb the warmup so Kc0's mesh starts immediately.
            sched_sb = const.tile([128, NPAIR], F32, name="sched_sb")
            nc.scalar.dma_start(dummy_in[:], sched_in[0:1, :])
            nc.gpsimd.collective_compute(
                "AllGather", ALU.bypass, replica_groups=RG,
                ins=[dummy_in.opt()], outs=[dummy_out.opt()],
            )
            nc.scalar.dma_start(sched_sb[:], sched_in)
            ident_sb = const.tile([128, 128], BF16, name="ident_sb")
            make_identity(nc, ident_sb[:])
            iota_sb = const.tile([128, BK], F32, name="iota_sb")
            nc.gpsimd.iota(
                iota_sb[:], pattern=[[-1, BK]], base=0, channel_multiplier=1,
                allow_small_or_imprecise_dtypes=True,
            )

            partials = const.tile([128, NPAIR], F32, name="partials")
            den = const.tile([128, 4], F32, name="den")
            rec = const.tile([128, 4], F32, name="rec")

            # ---------------- long-lived SBUF ----------------
            att = es.enter_context(tc.tile_pool(name="att", bufs=1))
            qt_sb = att.tile([128, 16, SB], BF16, name="qt_sb")

            qes = ExitStack()
            xqp = qes.enter_context(tc.tile_pool(name="xqp", bufs=1))
            xq_sb = xqp.tile([128, 16, SB], BF16, name="xq_sb")
            nc.gpsimd.dma_start(
                xq_sb[:], xq_in.rearrange("(k p) q -> p k q", p=128))
            stagep = qes.enter_context(tc.tile_pool(name="stagep", bufs=2))
            # Double-buffered full-weight pool: Wv -> buf0, Wk -> buf1,
            # Wq -> buf0 again (auto-dep: waits for V's last Wv read, ~80us;
            # Q only needs it at ~150us). Each weight loads as two half-tile
            # DMAs split over sync+scalar so descriptors spread across DMA
            # queues; emission order Wv, Wk, Wq sets the bandwidth priority.
            # Weight pool: six ki-half tiles rotating through 4 buffers.
            # wq halves rotate into wk's buffers (dep: K matmuls done).
            wpool = qes.enter_context(tc.tile_pool(name="wpool", bufs=4))

            def wh_tile(name):
                return wpool.tile([128, 8, D], BF16, tag="wh", name=name)

            def load_wh(w_sb, w_src, eng, hh):
                eng.dma_start(
                    w_sb[:],
                    w_src[1024 * hh:1024 * (hh + 1), :]
                    .rearrange("(k p) d -> p k d", p=128))

            # sync carries ONLY wk's first half, so its queue frees early
            # for the kq writes + attention loads.
            wkA = wh_tile("wkA"); wkB = wh_tile("wkB")
            wvA = wh_tile("wvA"); wvB = wh_tile("wvB")
            # wkA split across sync+gpsimd quarters: K's first matmuls are
            # gated on it, and one engine queue moves only ~160GB/s.
            nc.sync.dma_start(
                wkA[:, 0:4, :],
                wk_in[0:512, :].rearrange("(k p) d -> p k d", p=128))
            nc.gpsimd.dma_start(
                wkA[:, 4:8, :],
                wk_in[512:1024, :].rearrange("(k p) d -> p k d", p=128))
            load_wh(wkB, wk_in, nc.scalar, 1)
            load_wh(wvA, wv_in, nc.scalar, 0)
            load_wh(wvB, wv_in, nc.gpsimd, 1)

            # ------- phase K: local K^T (all d_out, own q), tile-major out,
            # two tile-group AllGathers (KAG0 fires here; KAG1 is emitted
            # after VAG0 in the V phase so the CC order is
            # KAG0, VAG0, KAG1, VAG1) -------
            if True:
                for dp in range(8):
                    psA = psum.tile([128, BK], F32, tag="pw", bufs=3,
                                    name=f"pskA{dp}")
                    psB = psum.tile([128, BK], F32, tag="pw", bufs=3,
                                    name=f"pskB{dp}")
                    for ki in range(16):
                        wkh = wkA if ki < 8 else wkB
                        nc.tensor.matmul(
                            psA[:, 0:SB], lhsT=wkh[:, ki % 8, 256 * dp:
                                                   256 * dp + 128],
                            rhs=xq_sb[:, ki, :],
                            start=(ki == 0), stop=(ki == 15),
                        )
                        nc.tensor.matmul(
                            psB[:, 0:SB], lhsT=wkh[:, ki % 8, 256 * dp + 128:
                                                   256 * dp + 256],
                            rhs=xq_sb[:, ki, :],
                            start=(ki == 0), stop=(ki == 15),
                        )
                    for half, ps in ((0, psA), (1, psB)):
                        dt = 2 * dp + half
                        kst = stagep.tile([128, SB], BF16, tag="kst",
                                          name=f"kst{dt}")
                        nc.vector.tensor_copy(kst[:], ps[:, 0:SB])
                        nc.sync.dma_start(
                            kq_in[dt // 8][128 * (dt % 8):
                                           128 * (dt % 8) + 128, :],
                            kst[:],
                        )
                    if dp in (3, 7):
                        qd = dp // 4
                        nc.gpsimd.collective_compute(
                            "AllGather", ALU.bypass, replica_groups=RG,
                            ins=[kq_in[qd].opt()], outs=[kg[qd].opt()],
                        )

            # Wq^T halves rotate into Wk^T's buffers (dep: K matmuls done;
            # Q needs them only after V).
            wqA = wh_tile("wqA"); wqB = wh_tile("wqB")
            load_wh(wqA, wq_in, nc.scalar, 0)
            load_wh(wqB, wq_in, nc.scalar, 1)

            # ------- phase V: local V projection (natural layout) ------
            if True:
                for st in range(4):
                    vq_dst = vq_in[st // 2].rearrange("(t p) d -> p t d",
                                                      p=128)
                    for h in range(2):
                        ps = psum.tile([128, BK], F32, tag="pw", bufs=3,
                                       name=f"psv{st}_{h}")
                        for ki in range(16):
                            for n2 in range(2):
                                nc.tensor.matmul(
                                    ps[:, 512 * n2:512 * (n2 + 1)],
                                    lhsT=xq_sb[:, ki, 128 * st:128 * (st + 1)],
                                    rhs=(wvA if ki < 8 else wvB)[
                                        :, ki % 8, BK * h + 512 * n2:
                                        BK * h + 512 * (n2 + 1)],
                                    start=(ki == 0), stop=(ki == 15),
                                    skip_group_check=True,
                                )
                        vst = stagep.tile([128, BK], BF16, tag="vst", bufs=2,
                                          name=f"vst{st}_{h}")
                        nc.vector.tensor_copy(vst[:], ps[:])
                        nc.gpsimd.dma_start(
                            vq_dst[:, st % 2, BK * h:BK * (h + 1)], vst[:])
                    if st == 1:
                        nc.gpsimd.collective_compute(
                            "AllGather", ALU.bypass, replica_groups=RG,
                            ins=[vq_in[0].opt()], outs=[vg[0].opt()],
                        )
                nc.gpsimd.collective_compute(
                    "AllGather", ALU.bypass, replica_groups=RG,
                    ins=[vq_in[1].opt()], outs=[vg[1].opt()],
                )

            # ------- phase Q: local Q^T projection (resident Wq^T) ------
            if True:
                for dp in range(8):
                    psA = psum.tile([128, BK], F32, tag="pw", bufs=3,
                                    name=f"psqA{dp}")
                    psB = psum.tile([128, BK], F32, tag="pw", bufs=3,
                                    name=f"psqB{dp}")
                    for ki in range(16):
                        wqh = wqA if ki < 8 else wqB
                        nc.tensor.matmul(
                            psA[:, 0:SB], lhsT=wqh[:, ki % 8, 256 * dp:
                                                   256 * dp + 128],
                            rhs=xq_sb[:, ki, :],
                            start=(ki == 0), stop=(ki == 15),
                        )
                        nc.tensor.matmul(
                            psB[:, 0:SB], lhsT=wqh[:, ki % 8, 256 * dp + 128:
                                                   256 * dp + 256],
                            rhs=xq_sb[:, ki, :],
                            start=(ki == 0), stop=(ki == 15),
                        )
                    nc.vector.tensor_copy(qt_sb[:, 2 * dp, :], psA[:, 0:SB])
                    nc.vector.tensor_copy(qt_sb[:, 2 * dp + 1, :],
                                          psB[:, 0:SB])
            qes.close()

            # ---------------- attention (software-pipelined) ----------------
            ktl = es.enter_context(tc.tile_pool(name="ktl", bufs=3))
            vtl = es.enter_context(tc.tile_pool(name="vtl", bufs=3))
            mkl = es.enter_context(tc.tile_pool(name="mkl", bufs=2))
            pwork = es.enter_context(tc.tile_pool(name="pwork", bufs=2))

            acc = [att.tile([128, D], F32, name=f"acc{t}") for t in range(4)]

            def normalize_slot(slot):
                nc.vector.tensor_reduce(
                    den[:, slot:slot + 1],
                    partials[:, PBASE[slot]:PBASE[slot] + KBMAX[slot]],
                    axis=mybir.AxisListType.X, op=ALU.add,
                )
                nc.vector.reciprocal(rec[:, slot:slot + 1],
                                     den[:, slot:slot + 1])
                nc.vector.tensor_scalar_mul(
                    acc[slot][:], acc[slot][:], rec[:, slot:slot + 1])
                nc.scalar.dma_start(
                    out_ext[128 * slot:128 * (slot + 1), :], acc[slot][:])

            def tp_stage(st):
                pm, vtA, vtB, B, slot = st
                pmt = pwork.tile([128, 8, 128], BF16, tag="pmt",
                                 name=f"pmt{B}_{slot}")
                for j in range(8):
                    tp = psum.tile([128, 128], BF16, tag="tp", bufs=2,
                                   name=f"tp{B}_{slot}_{j}")
                    nc.tensor.matmul(
                        tp[:], lhsT=pm[:, 128 * j:128 * (j + 1)],
                        rhs=ident_sb[:], is_transpose=True,
                        skip_group_check=True)
                    nc.scalar.copy(pmt[:, j, :], tp[:])
                return pmt

            def av_stage(st, pmt):
                pm, vtA, vtB, B, slot = st
                for h, vt in ((0, vtA), (1, vtB)):
                    av = psum.tile([128, BK], F32, tag="pw", bufs=3,
                                   name=f"av{B}_{slot}_{h}")
                    for j in range(8):
                        for n2 in range(2):
                            nc.tensor.matmul(
                                av[:, 512 * n2:512 * (n2 + 1)],
                                lhsT=pmt[:, j, :],
                                rhs=vt[:, j, 512 * n2:512 * (n2 + 1)],
                                start=(j == 0), stop=(j == 7),
                                skip_group_check=True,
                            )
                    if B == 0:
                        nc.vector.tensor_copy(
                            acc[slot][:, BK * h:BK * (h + 1)], av[:])
                    else:
                        nc.vector.scalar_tensor_tensor(
                            out=acc[slot][:, BK * h:BK * (h + 1)],
                            in0=av[:], scalar=1.0,
                            in1=acc[slot][:, BK * h:BK * (h + 1)],
                            op0=ALU.mult, op1=ALU.add,
                        )

            prev = None
            prev_pmt = None
            for B in range(NBIG):
                # kt halves: ktA = ki 0..7, ktB = ki 8..15 of K^T big-block B
                ktA = ktl.tile([128, 8, BK], BF16, tag="kt", name=f"ktA{B}")
                ktB = ktl.tile([128, 8, BK], BF16, tag="kt", name=f"ktB{B}")
                # chunk-major emission: each dout-half's 8 sub-loads flow
                # as soon as that half's AllGather lands.
                for H, kth in ((0, ktA), (1, ktB)):
                    for j in range(8):
                        t = 8 * B + j
                        c, s = tile_owner_slot(t)
                        nc.sync.dma_start(
                            kth[:, :, 128 * j:128 * (j + 1)],
                            kg[H][BK * c:BK * (c + 1),
                                  128 * s:128 * (s + 1)]
                            .rearrange("(k p) q -> p k q", p=128),
                        )
                vtA = vtl.tile([128, 8, BK], BF16, tag="vt", name=f"vtA{B}")
                vtB = vtl.tile([128, 8, BK], BF16, tag="vt", name=f"vtB{B}")
                for j in range(8):
                    t = 8 * B + j
                    vgx = vg[t // 16]
                    r0 = VROW2[t]
                    nc.sync.dma_start(vtA[:, j, :], vgx[r0:r0 + 128, 0:BK])
                    nc.sync.dma_start(vtB[:, j, :], vgx[r0:r0 + 128, BK:D])
                for slot in range(B, 4):
                    p = PBASE[slot] + B
                    mk = mkl.tile([128, BK], BF16, tag="mk",
                                  name=f"mk{B}_{slot}")
                    nc.scalar.dma_start(
                        mk[:],
                        mask_in[128 * slot:128 * (slot + 1),
                                BK * B:BK * (B + 1)],
                    )
                    sc = psum.tile([128, BK], F32, tag="pw", bufs=3,
                                   name=f"sc{B}_{slot}")
                    for ki in range(16):
                        if ki == 8 and prev is not None:
                            # interleave prev pair's P-transposes here so
                            # the pmt copies finish before its attn@V
                            prev_pmt = tp_stage(prev)
                        kth = ktA if ki < 8 else ktB
                        for n2 in range(2):
                            nc.tensor.matmul(
                                sc[:, 512 * n2:512 * (n2 + 1)],
                                lhsT=qt_sb[:, ki, 128 * slot:128 * (slot + 1)],
                                rhs=kth[:, ki % 8, 512 * n2:512 * (n2 + 1)],
                                start=(ki == 0), stop=(ki == 15),
                                skip_group_check=True,
                            )
                    pex = pwork.tile([128, BK], BF16, tag="pex", bufs=1,
                                     name=f"pex{B}_{slot}")
                    nc.scalar.activation(pex[:], sc[:], AFT.Exp, scale=SCALE)
                    pcs = pwork.tile([128, BK], BF16, tag="pcs", bufs=1,
                                     name=f"pcs{B}_{slot}")
                    nc.vector.scalar_tensor_tensor(
                        out=pcs[:], in0=iota_sb[:],
                        scalar=sched_sb[:, p:p + 1], in1=pex[:],
                        op0=ALU.is_ge, op1=ALU.mult,
                        accum_out=partials[:, p:p + 1],
                    )
                    pm = pwork.tile([128, BK], BF16, tag="pm",
                                    name=f"pm{B}_{slot}")
                    nc.vector.tensor_mul(pm[:], pcs[:], mk[:])
                    if prev is not None:
                        av_stage(prev, prev_pmt)
                    prev = (pm, vtA, vtB, B, slot)
            prev_pmt = tp_stage(prev)
            av_stage(prev, prev_pmt)
            for slot in range(4):
                normalize_slot(slot)

    nc.compile()
    return nc


_NC_CACHE = None


def _get_nc():
    global _NC_CACHE
    if _NC_CACHE is None:
        _NC_CACHE = build()
    return _NC_CACHE


def make_in_maps(x, Wq, Wk, Wv, drop_mask):
    bf = ml_dtypes.bfloat16
    x = np.asarray(x, dtype=np.float32)
    Wq = np.asarray(Wq, dtype=np.float32)
    Wk = np.asarray(Wk, dtype=np.float32)
    Wv = np.asarray(Wv, dtype=np.float32)
    drop_mask = np.asarray(drop_mask, dtype=np.float32)

    xT = np.ascontiguousarray(x.T).astype(bf)           # [D, S]
    wqT = np.ascontiguousarray(Wq.T.astype(bf))         # [D, D]
    wvT = np.ascontiguousarray(Wv.T.astype(bf))         # [D, D]
    wkT = np.ascontiguousarray(Wk.T.astype(bf))         # [D, D]
    mask_bf = drop_mask.astype(bf)

    in_maps = []
    for c in range(NC):
        tl = owned_tiles(c)
        thr = np.array(
            [1024.0 * B - 128.0 * tl[slot]
             for slot in range(4) for B in range(KBMAX[slot])],
            dtype=np.float32,
        )
        in_maps.append({
            "xq": np.ascontiguousarray(
                np.concatenate([xT[:, 128 * t:128 * (t + 1)] for t in tl],
                               axis=1)),
            "wqT": wqT,
            "wvT": wvT,
            "wkT": wkT,
            "drop_mask": np.ascontiguousarray(
                np.concatenate(
                    [mask_bf[128 * t:128 * (t + 1)] for t in tl], axis=0)),
            "sched": np.ascontiguousarray(np.tile(thr[None, :], (128, 1))),
        })
    return in_maps


def assemble(results):
    full = np.zeros((S, D), dtype=np.float32)
    for c in range(NC):
        o = results[c]["out"]
        for slot, t in enumerate(owned_tiles(c)):
            full[128 * t:128 * (t + 1)] = o[128 * slot:128 * (slot + 1)]
    return full


def kernel(x, Wq, Wk, Wv, drop_mask):
    nc = _get_nc()
    in_maps = make_in_maps(x, Wq, Wk, Wv, drop_mask)
    res = bass_utils.run_bass_kernel_spmd(nc, in_maps, core_ids=list(range(NC)))
    return assemble(res.results)


def kernel_profiled(x, Wq, Wk, Wv, drop_mask):
    """Like kernel(), but captures an NTFF profile; returns (out, exec_time_ns,
    trace_path)."""
    nc = _get_nc()
    in_maps = make_in_maps(x, Wq, Wk, Wv, drop_mask)
    res = bass_utils.run_bass_kernel_spmd(
        nc, in_maps, core_ids=list(range(NC)), trace=True)
    trace_path = None
    if res.instructions_and_trace is not None:
        trace_path = res.instructions_and_trace[1]
    return assemble(res.results), res.exec_time_ns, trace_path

